# revision 1
# baseline (speedup 1.0000x reference)
"""NeuromorphicBrainZone Trainium2 kernel (8 NeuronCores, Bass/Tile).

Math (per reference):
    x2 = x.reshape(T, D)                                     # T=1024, D=512
    zone[t, j] = b_in[j] - mean_d |x2[t, d] - W_in[j, d]|    # N=2048
    spikes     = sigmoid(SURR_BETA * (zone - v_th))
    out[t, m]  = b_out[m] - mean_j |spikes[t, j] - W_out[m, j]|

Sharding: the layer-1 neuron dim j is sharded 8 ways (256 j per core, all
tokens). Layer 2 reduces over j, so each core computes partial sums over
its local j for ALL (t, m); a ReduceScatter(add) over the cores both
completes the j-reduction and leaves each core an m-shard (64 rows) of
the output. The host stitches/transposes (free vs HW time).

On-core algorithm: the reduce dim (d for L1, j for L2) lives on SBUF
partitions. Using |x-w| = 2*max(x,w) - x - w, the elementwise work is a
single DVE tensor_scalar max(x, w) per (out-idx, reduce-block) in bf16
(exact: max of bf16 inputs picks one of them). The partition-reduction
runs on the PE: a matmul whose lhsT is a shifted ones-column window with
value +2 at column j accumulates 2*colsum(max-tile) into PSUM row j.
Two cheap block-level corrections complete the identity:
  - an all-(-1) lhsT streams the x (or spikes) tiles once per block,
    adding -sum_d x_d to every PSUM row (exact cancellation in bf16);
  - a host-built lhsT whose column j is -sum_d(W[j,:])/128, against an
    all-ones rhs, adds the -sum_d w_jd constant per row.
PSUM rows are evacuated by one fused ACT op per 128-row block
(sigmoid(scale*psum + beta) for L1, identity scale+bias for L2).
Optionally some reduce-blocks go to the ACT engine instead as a fused
Abs(x - w) activation (bias = -w column, +1 window, no corrections).
"""

import sys

sys.path.insert(0, "/opt/trn_rl_repo")

from contextlib import ExitStack

import numpy as np

import concourse.bass as bass
import concourse.bacc as bacc
import concourse.mybir as mybir
import concourse.tile as tile

SURR_BETA = 4.0
# reduce-blocks handled by ACT (fused abs) instead of DVE (2*max):
ACT1_DBS = ()   # layer-1 d-blocks (of 4)
ACT2_JBS = ()   # layer-2 j-blocks (of 2)


def build_kernel(n_cores=8, T=1024, D=512, N=2048, M=512,
                 act1_dbs=ACT1_DBS, act2_jbs=ACT2_JBS):
    JC = N // n_cores          # local neurons
    MS = M // n_cores          # output m-shard
    n_dblk = D // 128
    n_jblk = JC // 128
    n_mblk = M // 128
    CH = 512                   # matmul free-dim chunk (one PSUM bank)
    n_ch = (T + CH - 1) // CH
    bf16 = mybir.dt.bfloat16
    f32 = mybir.dt.float32
    Act = mybir.ActivationFunctionType
    dve1_dbs = [db for db in range(n_dblk) if db not in act1_dbs]
    dve2_jbs = [jb for jb in range(n_jblk) if jb not in act2_jbs]

    nc = bacc.Bacc("TRN2", target_bir_lowering=False, debug=False,
                   num_devices=n_cores)

    xT_d = nc.dram_tensor("xT", [D, T], bf16, kind="ExternalInput")
    negw1_d = nc.dram_tensor("negw1", [D, JC], f32, kind="ExternalInput")
    posw1_d = nc.dram_tensor("posw1", [D, JC], f32, kind="ExternalInput")
    beta_d = nc.dram_tensor("beta", [JC], f32, kind="ExternalInput")
    negw2_d = nc.dram_tensor("negw2", [JC, M], f32, kind="ExternalInput")
    posw2_d = nc.dram_tensor("posw2", [JC, M], f32, kind="ExternalInput")
    bo_d = nc.dram_tensor("bo", [M], f32, kind="ExternalInput")
    wd1_d = nc.dram_tensor("wd1", [JC, 128], bf16, kind="ExternalInput")
    wd2_d = nc.dram_tensor("wd2", [M, 128], bf16, kind="ExternalInput")
    out_d = nc.dram_tensor("out", [MS, T], f32, kind="ExternalOutput")

    with tile.TileContext(nc) as tc, ExitStack() as ctx:
        cpool = ctx.enter_context(tc.tile_pool(name="const", bufs=1))
        apool = ctx.enter_context(tc.tile_pool(name="abs", bufs=10))
        spool = ctx.enter_context(tc.tile_pool(name="spk", bufs=1))
        opool = ctx.enter_context(tc.tile_pool(name="out", bufs=1))
        ppool = ctx.enter_context(tc.tile_pool(name="psum", bufs=2, space="PSUM"))
        dpool = ctx.enter_context(tc.tile_pool(name="dram", bufs=1, space="DRAM"))

        # ---- constants / inputs to SBUF ----
        def load(name, src_ap, shape, dtype):
            t = cpool.tile(shape, dtype, tag=name, name=name)
            nc.sync.dma_start(t[:], src_ap)
            return t

        x_sb, negw1_sb, posw1_sb = [], [], []
        for db in range(n_dblk):
            r = slice(db * 128, (db + 1) * 128)
            x_sb.append(load(f"x{db}", xT_d[r, :], [128, T], bf16))
            negw1_sb.append(load(f"nw1{db}", negw1_d[r, :], [128, JC], f32))
            posw1_sb.append(load(f"pw1{db}", posw1_d[r, :], [128, JC], f32))
        negw2_sb, posw2_sb, beta_sb, wd1_sb, spikes = [], [], [], [], []
        beta2d = beta_d.ap().rearrange("(p o) -> p o", o=1)
        for jb in range(n_jblk):
            r = slice(jb * 128, (jb + 1) * 128)
            negw2_sb.append(load(f"nw2{jb}", negw2_d[r, :], [128, M], f32))
            posw2_sb.append(load(f"pw2{jb}", posw2_d[r, :], [128, M], f32))
            beta_sb.append(load(f"beta{jb}", beta2d[r, :], [128, 1], f32))
            wd1_sb.append(load(f"wd1{jb}", wd1_d[r, :], [128, 128], bf16))
            spikes.append(spool.tile([128, T], bf16, tag=f"spk{jb}",
                                     name=f"spk{jb}"))
        bo2d = bo_d.ap().rearrange("(p o) -> p o", o=1)
        bo_sb, wd2_sb = [], []
        for mb in range(n_mblk):
            r = slice(mb * 128, (mb + 1) * 128)
            bo_sb.append(load(f"bo{mb}", bo2d[r, :], [128, 1], f32))
            wd2_sb.append(load(f"wd2{mb}", wd2_d[r, :], [128, 128], bf16))
        partial_big = opool.tile([128, n_mblk * T], f32, tag="par", name="par")

        # window tensors: G*/H* have a single column of value v such that
        # window(j)[k, m] = v iff m == j. Separate even/odd tensors keep
        # the lhsT window starts 4-byte aligned.
        def winpair(name, v):
            g = cpool.tile([128, 256], bf16, tag=f"{name}g", name=f"{name}g")
            h = cpool.tile([128, 256], bf16, tag=f"{name}h", name=f"{name}h")
            nc.vector.memset(g[:], 0.0)
            nc.vector.memset(h[:], 0.0)
            nc.vector.memset(g[:, 128:129], v)
            nc.vector.memset(h[:, 127:128], v)
            return g, h

        G1, H1 = winpair("w1", 1.0)
        G2, H2 = winpair("w2", 2.0)
        negones = cpool.tile([128, 128], bf16, tag="negones", name="negones")
        nc.vector.memset(negones[:], -1.0)
        ones_rhs = cpool.tile([128, CH], bf16, tag="ones_rhs", name="ones_rhs")
        nc.vector.memset(ones_rhs[:], 1.0)

        def window(j, two):
            g, h = (G2, H2) if two else (G1, H1)
            if j % 2 == 0:
                return g[:, 128 - j:256 - j]
            return h[:, 127 - j:255 - j]

        def layer(n_out_blk, n_red_blk, act_rbs, dve_rbs, src_sb, pos_sb,
                  neg_sb, wd_sb, evac, first_tiles=None):
            """One L1-distance layer: for each 128-row output block,
            accumulate sum_red |src - w| into PSUM rows and evacuate."""
            for ob in range(n_out_blk):
                psum = ppool.tile([128, T], f32, tag="ps", name="ps")
                for oo in range(128):
                    o = ob * 128 + oo
                    for rb in range(n_red_blk):
                        if first_tiles and ob == 0 and oo == 0 and rb < len(first_tiles):
                            a = first_tiles[rb]
                        else:
                            a = apool.tile([128, T], bf16, tag="abs", name="ab")
                        if rb in act_rbs:
                            nc.scalar.activation(a[:], src_sb[rb][:], Act.Abs,
                                                 bias=neg_sb[rb][:, o:o + 1],
                                                 scale=1.0)
                            win = window(oo, two=False)
                        else:
                            nc.vector.tensor_scalar(
                                a[:], src_sb[rb][:], pos_sb[rb][:, o:o + 1],
                                None, op0=mybir.AluOpType.max)
                            win = window(oo, two=True)
                        unit_last = (not dve_rbs and oo == 127
                                     and rb == n_red_blk - 1)
                        for c in range(n_ch):
                            nc.tensor.matmul(
                                psum[:, c * CH:(c + 1) * CH], win,
                                a[:, c * CH:(c + 1) * CH],
                                start=(oo == 0 and rb == 0),
                                stop=(unit_last and c == n_ch - 1))
                # corrections for the 2*max identity (DVE blocks only):
                # -sum_red src into every row, then -sum_red w per row.
                for rb in dve_rbs:
                    for c in range(n_ch):
                        nc.tensor.matmul(
                            psum[:, c * CH:(c + 1) * CH], negones[:, :],
                            src_sb[rb][:, c * CH:(c + 1) * CH],
                            start=False, stop=False)
                if dve_rbs:
                    for c in range(n_ch):
                        nc.tensor.matmul(
                            psum[:, c * CH:(c + 1) * CH], wd_sb[ob][:, :],
                            ones_rhs[:, :CH],
                            start=False, stop=(c == n_ch - 1))
                evac(ob, psum)

        # ---- layer 1 -> spikes ----
        def evac1(jb, psum):
            nc.scalar.activation(spikes[jb][:], psum[:], Act.Sigmoid,
                                 bias=beta_sb[jb][:, 0:1],
                                 scale=-SURR_BETA / D)

        layer(n_jblk, n_dblk, act1_dbs, dve1_dbs, x_sb, posw1_sb, negw1_sb,
              wd1_sb, evac1)

        # ---- layer 2 -> partial output ----
        # First two L2 units use dedicated tiles: pooled slots would add
        # PE+DVE release waits on top of ACT(spikes)+DMA deps.
        l2first = [cpool.tile([128, T], bf16, tag=f"l2f{i}", name=f"l2f{i}")
                   for i in range(min(2, n_jblk))]

        def evac2(mb, psum):
            nc.scalar.activation(partial_big[:, mb * T:(mb + 1) * T], psum[:],
                                 Act.Identity,
                                 bias=bo_sb[mb][:, 0:1], scale=-1.0 / N)

        layer(n_mblk, n_jblk, act2_jbs, dve2_jbs, spikes, posw2_sb, negw2_sb,
              wd2_sb, evac2, first_tiles=l2first)

        # ---- ReduceScatter over cores -> local m-shard ----
        bounce_in = dpool.tile([M, T], f32, tag="cin", name="cin")
        bounce_out = dpool.tile([MS, T], f32, tag="cout", name="cout")
        nc.sync.dma_start(
            bounce_in.rearrange("(mb p) t -> p mb t", p=128),
            partial_big.rearrange("p (mb t) -> p mb t", t=T))
        nc.gpsimd.collective_compute(
            "ReduceScatter",
            mybir.AluOpType.add,
            replica_groups=[list(range(n_cores))],
            ins=[bounce_in.opt()],
            outs=[bounce_out.opt()],
        )
        nc.sync.dma_start(out_d[:, :], bounce_out[:])

    nc.compile()
    return nc


def prep_inputs(x, W_in, b_in, W_out, b_out, v_th, n_cores=8,
                act1_dbs=ACT1_DBS, act2_jbs=ACT2_JBS):
    """Host-side prep: transposes, negation, W-sum folding. Per-core maps."""
    import ml_dtypes

    bf16 = ml_dtypes.bfloat16
    T = x.shape[0] * x.shape[1]
    D = x.shape[2]
    N = W_in.shape[0]
    M = W_out.shape[0]
    JC = N // n_cores
    n_dblk = D // 128
    n_jblk = JC // 128
    n_mblk = M // 128

    xT = np.ascontiguousarray(x.reshape(T, D).T).astype(bf16)
    w1T = np.ascontiguousarray(W_in.T.astype(np.float32))        # [D, N]
    beta = (SURR_BETA * (b_in - v_th)).astype(np.float32)        # [N]
    w2T = np.ascontiguousarray(W_out.T.astype(np.float32))       # [N, M]
    bo = (b_out / n_cores).astype(np.float32)                    # [M]

    # W-sum folds for the 2*max corrections, restricted to DVE blocks.
    # Sums are taken over the bf16-rounded weights the device actually
    # sees (the max-op compares against f32 w, but the correction matrix
    # is bf16; use f32 sums of f32 weights - bf16 rounding of wd matters
    # more and is divided by 128 anyway).
    dve1 = [db for db in range(n_dblk) if db not in act1_dbs]
    dve2 = [jb for jb in range(n_jblk) if jb not in act2_jbs]
    dmask = np.zeros(D, bool)
    for db in dve1:
        dmask[db * 128:(db + 1) * 128] = True
    wsum1 = W_in[:, dmask].sum(1).astype(np.float32)             # [N]

    in_maps = []
    for c in range(n_cores):
        sl = slice(c * JC, (c + 1) * JC)
        jmask = np.zeros(JC, bool)
        for jb in dve2:
            jmask[jb * 128:(jb + 1) * 128] = True
        # wd1: per local-j block, [128, 128] matrix, col jj = -wsum1[j]/128
        wd1_blocks = np.concatenate(
            [np.broadcast_to((-wsum1[sl][jb * 128:(jb + 1) * 128] / 128.0)[None, :],
                             (128, 128)) for jb in range(n_jblk)], axis=0)
        wsum2 = W_out[:, c * JC:(c + 1) * JC][:, jmask].sum(1)   # [M]
        wd2_blocks = np.concatenate(
            [np.broadcast_to((-wsum2[mb * 128:(mb + 1) * 128] / 128.0)[None, :],
                             (128, 128)) for mb in range(n_mblk)], axis=0)
        in_maps.append({
            "xT": xT,
            "negw1": np.ascontiguousarray(-w1T[:, sl]),
            "posw1": np.ascontiguousarray(w1T[:, sl]),
            "beta": np.ascontiguousarray(beta[sl]),
            "negw2": np.ascontiguousarray(-w2T[sl, :]),
            "posw2": np.ascontiguousarray(w2T[sl, :]),
            "bo": bo,
            "wd1": np.ascontiguousarray(wd1_blocks).astype(bf16),
            "wd2": np.ascontiguousarray(wd2_blocks).astype(bf16),
        })
    return in_maps


_NC_CACHE = {}


def _get_nc():
    if "nc" not in _NC_CACHE:
        _NC_CACHE["nc"] = build_kernel()
    return _NC_CACHE["nc"]


def run_on_hw(inputs, trace=False, tmpdir=None):
    """Run on the 8 NeuronCores; returns (full_output, BassKernelResults)."""
    from concourse.bass_utils import run_bass_kernel_spmd

    n_cores = 8
    nc = _get_nc()
    in_maps = prep_inputs(**inputs, n_cores=n_cores)
    res = run_bass_kernel_spmd(nc, in_maps, core_ids=list(range(n_cores)),
                               trace=trace, tmpdir=tmpdir)
    B, S, D_model = inputs["x"].shape
    T = B * S
    M = inputs["W_out"].shape[0]
    MS = M // n_cores
    full = np.empty((M, T), np.float32)
    for c in range(n_cores):
        full[c * MS:(c + 1) * MS, :] = res.results[c]["out"]
    out = np.ascontiguousarray(full.T).reshape(B, S, D_model).astype(np.float32)
    return out, res


def kernel(x, W_in, b_in, W_out, b_out, v_th):
    out, _ = run_on_hw(dict(x=x, W_in=W_in, b_in=b_in, W_out=W_out,
                            b_out=b_out, v_th=v_th))
    return out



# revision 3
# speedup vs baseline: 5.1908x; 5.1908x over previous
"""NeuromorphicBrainZone Trainium2 kernel (8 NeuronCores, Bass/Tile).

Math (per reference):
    x2 = x.reshape(T, D)                                     # T=1024, D=512
    zone[t, j] = b_in[j] - mean_d |x2[t, d] - W_in[j, d]|    # N=2048
    spikes     = sigmoid(SURR_BETA * (zone - v_th))
    out[t, m]  = b_out[m] - mean_j |spikes[t, j] - W_out[m, j]|

Sharding: layer-1 neuron dim j sharded 8 ways (JC=256 j per core, all
tokens). Layer 2 reduces over j, so each core computes partials over its
local j for ALL (t, m); a chunked ReduceScatter(add) completes the
j-reduction and leaves each core m-shards of the output.

Algorithm (weight quantization -> real matmuls):
Each weight column (reduce-dim index r) is quantized to K levels
theta_k[r] (per-column quantiles, rounded to the compute dtype). Then
    |x - wq| = 2*max(x, theta_q) - x - wq
and, with S_k[r, j] = 2 * 1[quant(w_jr) = k] a 0/2 selection matrix,
    sum_r 2*max(x_tr, wq_jr) = sum_k (S_k^T @ M_k)[j, t],
    M_k[r, t] = max(x_rt, theta_k[r]).
So the per-(t,r) elementwise work is K tensor_scalar max tiles
(independent of the output count!), and the reduction is K dense
PE matmuls per output block. The -sum_r x term is an all-(-1) lhsT
matmul streaming the x (or spikes) tiles; the -sum_r wq term folds into
the evacuation bias on the host. Errors come only from weight
quantization and average out over the 512/2048-wide reductions
(measured ~1.1e-3 rel for K=16 in bf16).
"""

import sys

sys.path.insert(0, "/opt/trn_rl_repo")

from contextlib import ExitStack

import numpy as np

import concourse.bass as bass
import concourse.bacc as bacc
import concourse.mybir as mybir
import concourse.tile as tile

SURR_BETA = 4.0
K = 16                      # quantization levels per weight column
N_CORES = 8
T, D, N, M = 1024, 512, 2048, 512
JC = N // N_CORES           # local neurons (L1 outputs per core)
MS = M // N_CORES           # final output m-shard rows per core
N_DBLK = D // 128           # 4
N_JBLK = JC // 128          # 2
N_MBLK = M // 128           # 4
CH = 512                    # matmul free-dim chunk (one PSUM bank)
N_CH = T // CH              # 2


def build_kernel():
    bf16 = mybir.dt.bfloat16
    f32 = mybir.dt.float32
    DT = bf16
    Act = mybir.ActivationFunctionType

    nc = bacc.Bacc("TRN2", target_bir_lowering=False, debug=False,
                   num_devices=N_CORES)

    xp_d = nc.dram_tensor("xp", [128, N_DBLK * T], DT, kind="ExternalInput")
    th1_d = nc.dram_tensor("th1", [128, N_DBLK * K], f32, kind="ExternalInput")
    s1_d = nc.dram_tensor("s1", [K * 128, N_DBLK * JC], DT,
                          kind="ExternalInput")
    beta_d = nc.dram_tensor("beta", [128, N_JBLK], f32, kind="ExternalInput")
    th2_d = nc.dram_tensor("th2", [128, N_JBLK * K], f32, kind="ExternalInput")
    s2_d = nc.dram_tensor("s2", [K * 128, N_JBLK * M], DT,
                          kind="ExternalInput")
    bo_d = nc.dram_tensor("bo", [128, N_MBLK], f32, kind="ExternalInput")
    out_d = nc.dram_tensor("out", [MS, T], f32, kind="ExternalOutput")

    with tile.TileContext(nc) as tc, ExitStack() as ctx:
        cpool = ctx.enter_context(tc.tile_pool(name="const", bufs=1))
        mpool = ctx.enter_context(tc.tile_pool(name="m", bufs=6))
        ppool = ctx.enter_context(tc.tile_pool(name="psum", bufs=4,
                                               space="PSUM"))
        dpool = ctx.enter_context(tc.tile_pool(name="dram", bufs=1,
                                               space="DRAM"))

        def load(name, src, shape, dtype):
            t = cpool.tile(shape, dtype, tag=name, name=name)
            nc.sync.dma_start(t[:], src)
            return t

        xp = load("xp", xp_d[:, :], [128, N_DBLK * T], DT)
        th1 = load("th1", th1_d[:, :], [128, N_DBLK * K], f32)
        beta = load("beta", beta_d[:, :], [128, N_JBLK], f32)
        th2 = load("th2", th2_d[:, :], [128, N_JBLK * K], f32)
        bo = load("bo", bo_d[:, :], [128, N_MBLK], f32)
        s1 = [load(f"s1_{k}", s1_d[k * 128:(k + 1) * 128, :],
                   [128, N_DBLK * JC], DT) for k in range(K)]
        s2 = [load(f"s2_{k}", s2_d[k * 128:(k + 1) * 128, :],
                   [128, N_JBLK * M], DT) for k in range(K)]

        negones = cpool.tile([128, 128], DT, tag="negones", name="negones")
        nc.vector.memset(negones[:], -1.0)
        spikes = cpool.tile([128, N_JBLK * T], DT, tag="spk", name="spk")
        m2s = [[cpool.tile([128, T], DT, tag=f"m2_{k}_{jb}",
                           name=f"m2_{k}_{jb}") for jb in range(N_JBLK)]
               for k in range(K)]
        partial = [cpool.tile([128, T], f32, tag=f"par{mb}", name=f"par{mb}")
                   for mb in range(N_MBLK)]

        # ---- layer 1: psum1[jb][j, t] = 2*sum_d max(x, wq) - sum_d x ----
        ps1 = [ppool.tile([128, T], f32, tag="ps", name=f"ps1_{jb}")
               for jb in range(N_JBLK)]
        for k in range(K):
            for db in range(N_DBLK):
                m = mpool.tile([128, T], DT, tag="m1", name="m1")
                nc.vector.tensor_scalar(
                    m[:], xp[:, db * T:(db + 1) * T],
                    th1[:, db * K + k:db * K + k + 1], None,
                    op0=mybir.AluOpType.max)
                for jb in range(N_JBLK):
                    o = db * JC + jb * 128
                    for c in range(N_CH):
                        nc.tensor.matmul(
                            ps1[jb][:, c * CH:(c + 1) * CH],
                            s1[k][:, o:o + 128],
                            m[:, c * CH:(c + 1) * CH],
                            start=(k == 0 and db == 0), stop=False)
        for jb in range(N_JBLK):
            for db in range(N_DBLK):
                for c in range(N_CH):
                    nc.tensor.matmul(
                        ps1[jb][:, c * CH:(c + 1) * CH], negones[:, :],
                        xp[:, db * T + c * CH:db * T + c * CH + CH],
                        start=False,
                        stop=(db == N_DBLK - 1 and c == N_CH - 1))
            nc.scalar.activation(spikes[:, jb * T:(jb + 1) * T], ps1[jb][:],
                                 Act.Sigmoid, bias=beta[:, jb:jb + 1],
                                 scale=-SURR_BETA / D)

        # ---- layer 2: m2 tiles, then per-mblock accumulate + chunked RS ----
        for k in range(K):
            for jb in range(N_JBLK):
                nc.vector.tensor_scalar(
                    m2s[k][jb][:], spikes[:, jb * T:(jb + 1) * T],
                    th2[:, jb * K + k:jb * K + k + 1], None,
                    op0=mybir.AluOpType.max)
        for mb in range(N_MBLK):
            ps2 = ppool.tile([128, T], f32, tag="ps", name=f"ps2_{mb}")
            for k in range(K):
                for jb in range(N_JBLK):
                    o = jb * M + mb * 128
                    for c in range(N_CH):
                        nc.tensor.matmul(
                            ps2[:, c * CH:(c + 1) * CH],
                            s2[k][:, o:o + 128],
                            m2s[k][jb][:, c * CH:(c + 1) * CH],
                            start=(k == 0 and jb == 0), stop=False)
            for jb in range(N_JBLK):
                for c in range(N_CH):
                    nc.tensor.matmul(
                        ps2[:, c * CH:(c + 1) * CH], negones[:, :],
                        spikes[:, jb * T + c * CH:jb * T + c * CH + CH],
                        start=False,
                        stop=(jb == N_JBLK - 1 and c == N_CH - 1))
            nc.scalar.activation(partial[mb][:], ps2[:], Act.Identity,
                                 bias=bo[:, mb:mb + 1], scale=-1.0 / N)
            cin = dpool.tile([128, T], f32, tag=f"cin{mb}", name=f"cin{mb}")
            cout = dpool.tile([MS // N_MBLK, T], f32, tag=f"cout{mb}",
                              name=f"cout{mb}")
            nc.sync.dma_start(cin[:], partial[mb][:])
            nc.gpsimd.collective_compute(
                "ReduceScatter", mybir.AluOpType.add,
                replica_groups=[list(range(N_CORES))],
                ins=[cin.opt()], outs=[cout.opt()])
            nc.sync.dma_start(
                out_d[mb * (MS // N_MBLK):(mb + 1) * (MS // N_MBLK), :],
                cout[:])

    nc.compile()
    return nc


def _quant_cols(Wc, K, dt):
    """Per-column quantization of Wc [n_out, n_red] to K levels.
    Returns levels [n_red, K] (f32, dt-representable), idx [n_out, n_red],
    Wq [n_out, n_red] (f32 values of the quantized weights)."""
    qs = (np.arange(K, dtype=np.float64) + 0.5) / K
    lv = np.quantile(Wc.astype(np.float64), qs, axis=0).T
    lv = lv.astype(dt).astype(np.float32)                    # [n_red, K]
    idx = np.abs(Wc.T[:, :, None] - lv[:, None, :]).argmin(axis=2)
    Wq = lv[np.arange(lv.shape[0])[:, None], idx]            # [n_red, n_out]
    return lv, idx.T, Wq.T


def prep_inputs(x, W_in, b_in, W_out, b_out, v_th, n_cores=N_CORES):
    """Host-side prep: pack/transpose + weight quantization per core."""
    import ml_dtypes

    npdt = ml_dtypes.bfloat16

    x2 = np.asarray(x, np.float32).reshape(T, D)
    xT = np.ascontiguousarray(x2.T).astype(npdt)             # [D, T]
    xp = np.ascontiguousarray(
        xT.reshape(N_DBLK, 128, T).transpose(1, 0, 2).reshape(128, N_DBLK * T))

    # L1 quantization is global (same W_in for every core).
    lv1, idx1, Wq1 = _quant_cols(np.asarray(W_in, np.float32), K, npdt)
    # th1 packed [128, db*K + k]
    th1 = np.ascontiguousarray(
        lv1.reshape(N_DBLK, 128, K).transpose(1, 0, 2).reshape(128, N_DBLK * K)
    ).astype(np.float32)
    wq1_sum = Wq1.astype(np.float64).sum(axis=1)             # [N]
    beta_full = (SURR_BETA * (np.asarray(b_in, np.float64)
                              - np.asarray(v_th, np.float64)
                              + wq1_sum / D)).astype(np.float32)

    in_maps = []
    for c in range(n_cores):
        sl = slice(c * JC, (c + 1) * JC)
        # S1[k][d, j_local]: 2.0 where idx1 == k; packed rows k*128+p,
        # cols db*JC + j.
        idx1_loc = idx1[sl, :]                               # [JC, D]
        s1 = np.zeros((K, 128, N_DBLK * JC), np.float32)
        for db in range(N_DBLK):
            blk = idx1_loc[:, db * 128:(db + 1) * 128].T     # [128(d), JC]
            for k in range(K):
                s1[k, :, db * JC:(db + 1) * JC] = 2.0 * (blk == k)
        s1 = s1.reshape(K * 128, N_DBLK * JC).astype(npdt)

        beta = np.ascontiguousarray(
            beta_full[sl].reshape(N_JBLK, 128).T).astype(np.float32)

        # L2: quantize this core's W_out column slice per local-j column.
        W2c = np.asarray(W_out, np.float32)[:, sl]           # [M, JC]
        lv2, idx2, Wq2 = _quant_cols(W2c, K, npdt)           # lv2 [JC, K]
        th2 = np.ascontiguousarray(
            lv2.reshape(N_JBLK, 128, K).transpose(1, 0, 2)
            .reshape(128, N_JBLK * K)).astype(np.float32)
        # S2[k][j_local, m]: packed rows k*128+p (p = j within block),
        # cols jb*M + m.
        s2 = np.zeros((K, 128, N_JBLK * M), np.float32)
        for jb in range(N_JBLK):
            blk = idx2[:, jb * 128:(jb + 1) * 128]           # [M, 128(j)]
            for k in range(K):
                s2[k, :, jb * M:(jb + 1) * M] = 2.0 * (blk == k).T
        s2 = s2.reshape(K * 128, N_JBLK * M).astype(npdt)

        wq2_sum = Wq2.astype(np.float64).sum(axis=1)         # [M]
        bo_full = (np.asarray(b_out, np.float64) / n_cores
                   + wq2_sum / N).astype(np.float32)
        bo = np.ascontiguousarray(
            bo_full.reshape(N_MBLK, 128).T).astype(np.float32)

        in_maps.append({
            "xp": xp, "th1": th1, "s1": s1, "beta": beta,
            "th2": th2, "s2": s2, "bo": bo,
        })
    return in_maps


_NC_CACHE = {}


def _get_nc():
    if "nc" not in _NC_CACHE:
        _NC_CACHE["nc"] = build_kernel()
    return _NC_CACHE["nc"]


def run_on_hw(inputs, trace=False, tmpdir=None):
    """Run on the 8 NeuronCores; returns (full_output, BassKernelResults)."""
    from concourse.bass_utils import run_bass_kernel_spmd

    nc = _get_nc()
    in_maps = prep_inputs(**inputs)
    res = run_bass_kernel_spmd(nc, in_maps, core_ids=list(range(N_CORES)),
                               trace=trace, tmpdir=tmpdir)
    B, S, D_model = inputs["x"].shape
    # Core c's out rows (mb*MSB + r) hold m = mb*128 + c*MSB + r.
    MSB = MS // N_MBLK                                       # 16
    full = np.empty((M, T), np.float32)
    for c in range(N_CORES):
        o = res.results[c]["out"]
        for mb in range(N_MBLK):
            full[mb * 128 + c * MSB: mb * 128 + (c + 1) * MSB, :] = \
                o[mb * MSB:(mb + 1) * MSB, :]
    out = np.ascontiguousarray(full.T).reshape(B, S, D_model)
    return out.astype(np.float32), res


def kernel(x, W_in, b_in, W_out, b_out, v_th):
    out, _ = run_on_hw(dict(x=x, W_in=W_in, b_in=b_in, W_out=W_out,
                            b_out=b_out, v_th=v_th))
    return out
